# revision 6
# baseline (speedup 1.0000x reference)
"""Depthwise 3D conv (3x3x3, SAME, C=64) on 8 Trainium2 NeuronCores.

Strategy
--------
Data-parallel over (batch, d-half): core k handles b = k//2 and output
frames d in [8*(k%2), 8*(k%2)+8). The d-halo (1 frame each side) is
zero-padded on host so every core runs the identical 10-input-frame
program (SAME padding at batch edges falls out of the zero frames).

Per channel c the 27 taps factor into 9 TensorE matmuls, one per
(kd, kw): contraction over h_in with a per-channel banded matrix
B[h_in, h_out] that carries the 3 kh taps on its diagonals, PSUM-
accumulating all 9 into one [h=112, (d=4, w=112)] tile. The kd/kw
shifts are plain access-pattern offsets on the moving operand. Band
matrices are built on host (they're just w values on 3 diagonals) and
DMA'd in bf16; x is host-transposed to [h, c, d, w] bf16 (h outermost
so chunked DMAs get multi-KB contiguous runs per partition). Output
fp32.
"""

import json
import sys
import types

if "/opt/trn_rl_repo" not in sys.path:
    sys.path.insert(0, "/opt/trn_rl_repo")

import ml_dtypes
import numpy as np

KD = KH = KW = 3
C = 64
B_FULL, D_FULL, H, W = 4, 16, 112, 112
N_CORES = 8
D_OUT = 8  # output frames per core
D_IN = D_OUT + 2  # with zero-padded halo
DBLK = 4  # output frames per psum accumulation group
N_DBLK = D_OUT // DBLK
CG = 8  # channels per input DMA chunk
OG = 4  # channels per output DMA chunk
BF16 = ml_dtypes.bfloat16

_TAPS = [(0, 1), (0, 0), (0, 2), (1, 0), (1, 1), (1, 2), (2, 0), (2, 1), (2, 2)]


def _legalize_bir(raw: bytes) -> bytes:
    """walrus in this image caps sem waits at 1 per instruction; hoist extra
    waits onto preceding same-engine NoOps (sequencers run them in order)."""
    d = json.loads(raw)
    for fn in d["functions"]:
        for blk in fn["blocks"]:
            out = []
            for inst in blk["instructions"]:
                si = inst.get("sync_info")
                waits = (si or {}).get("on_wait") or []
                if len(waits) > 1:
                    for j, wt in enumerate(waits[:-1]):
                        out.append(
                            {
                                "debug": inst.get("debug", 0),
                                "engine": inst["engine"],
                                "ins": [],
                                "outs": [],
                                "name": f"{inst['name']}-w{j}",
                                "opcode": "NoOp",
                                "sync_info": {"on_wait": [wt], "on_update": []},
                            }
                        )
                    si["on_wait"] = [waits[-1]]
                out.append(inst)
            blk["instructions"] = out
    return json.dumps(d).encode()


def _build_nc():
    import concourse.bass as bass
    import concourse.mybir as mybir
    import concourse.tile as tile

    nc = bass.Bass()
    x_d = nc.declare_dram_parameter(
        "x", [H, C, D_IN, W], mybir.dt.bfloat16, isOutput=False
    )
    b_d = nc.declare_dram_parameter(
        "bmat", [H, C, len(_TAPS), H], mybir.dt.bfloat16, isOutput=False
    )
    y_d = nc.declare_dram_parameter("y", [H, C, D_OUT, W], mybir.dt.float32, isOutput=True)

    with tile.TileContext(nc) as tc:
        with (
            tc.tile_pool(name="xin", bufs=3) as xin_pool,
            tc.tile_pool(name="bmat", bufs=3) as b_pool,
            tc.tile_pool(name="psum", bufs=4, space="PSUM") as psum_pool,
            tc.tile_pool(name="osb", bufs=3) as osb_pool,
        ):
            for cg in range(C // CG):
                xt = xin_pool.tile([H, CG, D_IN, W], mybir.dt.bfloat16)
                bt = b_pool.tile([H, CG, len(_TAPS), H], mybir.dt.bfloat16)
                nc.sync.dma_start(out=xt[:], in_=x_d[:, cg * CG : (cg + 1) * CG])
                nc.sync.dma_start(out=bt[:], in_=b_d[:, cg * CG : (cg + 1) * CG])
                for oi in range(CG // OG):
                    osb = osb_pool.tile([H, OG, D_OUT, W], mybir.dt.float32)
                    for ci in range(OG):
                        cc = oi * OG + ci
                        for db in range(N_DBLK):
                            ps = psum_pool.tile([H, DBLK, W], mybir.dt.float32)
                            for i, (kd, kw) in enumerate(_TAPS):
                                d_lo = db * DBLK + kd
                                if kw == 1:
                                    wi, wj, wo, wp = 0, W, 0, W
                                elif kw == 0:
                                    wi, wj, wo, wp = 0, W - 1, 1, W
                                else:
                                    wi, wj, wo, wp = 1, W, 0, W - 1
                                t = kd * 3 + kw
                                nc.tensor.matmul(
                                    ps[:, :, wo:wp],
                                    bt[:, cc, t, :],
                                    xt[:, cc, d_lo : d_lo + DBLK, wi:wj],
                                    start=(i == 0),
                                    stop=(i == len(_TAPS) - 1),
                                    skip_group_check=(i != 0),
                                )
                            nc.scalar.copy(
                                out=osb[:, ci, db * DBLK : (db + 1) * DBLK, :],
                                in_=ps[:],
                            )
                    c0 = cg * CG + oi * OG
                    nc.sync.dma_start(out=y_d[:, c0 : c0 + OG], in_=osb[:])

    orig_to_json = nc.to_json_bytes
    nc.to_json_bytes = types.MethodType(
        lambda self: _legalize_bir(orig_to_json()), nc
    )
    return nc


def _host_prep(x: np.ndarray, w: np.ndarray):
    """Build per-core [h, c, d, w] bf16 inputs and the band matrices."""
    # x: (4, 16, 112, 112, 64) f32; w: (3, 3, 3, 1, 64) f32
    xt = np.ascontiguousarray(np.transpose(x, (0, 2, 4, 1, 3)))  # (b, h, c, d, w)

    wt = w[:, :, :, 0, :].astype(np.float32)  # (kd, kh, kw, c)
    bmat = np.zeros((H, C, len(_TAPS), H), np.float32)
    ho = np.arange(H)
    for kd in range(KD):
        for kw in range(KW):
            t = kd * 3 + kw
            for kh in range(KH):
                sel = ho[(ho + kh - 1 >= 0) & (ho + kh - 1 < H)]
                bmat[sel + kh - 1, :, t, sel] = wt[kd, kh, kw, :]
    bmat = bmat.astype(BF16)

    in_maps = []
    for k in range(N_CORES):
        b = k // 2
        d0 = (k % 2) * D_OUT
        lo, hi = d0 - 1, d0 + D_OUT + 1
        clo, chi = max(lo, 0), min(hi, D_FULL)
        xc = np.zeros((H, C, D_IN, W), BF16)
        xc[:, :, clo - lo : clo - lo + (chi - clo), :] = xt[b, :, :, clo:chi, :].astype(
            BF16
        )
        in_maps.append({"x": xc, "bmat": bmat})
    return in_maps


def _assemble(results):
    y = np.empty((B_FULL, D_FULL, H, W, C), np.float32)
    for k in range(N_CORES):
        b = k // 2
        d0 = (k % 2) * D_OUT
        # y core layout: (h, c, d, w) -> (d, h, w, c)
        y[b, d0 : d0 + D_OUT] = np.transpose(results[k]["y"], (2, 0, 3, 1))
    return y


def _run(x: np.ndarray, w: np.ndarray, trace: bool = False):
    from concourse.bass_utils import run_bass_kernel_spmd

    in_maps = _host_prep(np.asarray(x), np.asarray(w))
    nc = _build_nc()
    res = run_bass_kernel_spmd(nc, in_maps, list(range(N_CORES)), trace=trace)
    return _assemble(res.results), res.exec_time_ns


def kernel(x: np.ndarray, w: np.ndarray) -> np.ndarray:
    y, _ = _run(x, w, trace=False)
    return y
